# revision 1
# baseline (speedup 1.0000x reference)
"""Distributed causal attention kernel for Trainium2 (8 NeuronCores).

Tensor-parallel over heads (per sharding hint): core c owns heads {2c, 2c+1}.
Per core: QKV projection (with host-permuted Wproj rows so RoPE becomes
contiguous-partition elementwise ops), causal attention in the S^T
(keys-on-partitions) layout with a ones-column rowsum trick, then an
AllToAll that re-shards Y^T from head-sharded to sequence-sharded, and a
final output projection of this core's 512-row block. Host concatenates.
"""
import sys

if "/opt/trn_rl_repo" not in sys.path:
    sys.path.insert(0, "/opt/trn_rl_repo")

import math
from contextlib import ExitStack

import numpy as np
import ml_dtypes

import concourse.bass as bass
import concourse.bacc as bacc
import concourse.tile as tile
import concourse.mybir as mybir
from concourse.bass_utils import run_bass_kernel_spmd
from concourse.masks import make_identity

BF16 = mybir.dt.bfloat16
F32 = mybir.dt.float32

N_CORES = 8
S = 4096
D = 1024
H = 16
DH = 64
HPC = H // N_CORES          # heads per core = 2
W = 512                     # seq window
NW = S // W                 # windows
ROWS = S // N_CORES         # output rows per core = 512


def build_graph():
    nc = bacc.Bacc("TRN2", target_bir_lowering=False, debug=False,
                   num_devices=N_CORES)

    x_in = nc.dram_tensor("x", [S, D], BF16, kind="ExternalInput").ap()
    wt_in = nc.dram_tensor("wt", [D, 3 * HPC * DH], BF16, kind="ExternalInput").ap()
    wot_in = nc.dram_tensor("wot", [D, D], BF16, kind="ExternalInput").ap()
    cos_in = nc.dram_tensor("cos", [128, S], BF16, kind="ExternalInput").ap()
    sin_in = nc.dram_tensor("sin", [128, S], BF16, kind="ExternalInput").ap()
    msk_in = nc.dram_tensor("msk", [128, 4 * 2 * W], BF16, kind="ExternalInput").ap()
    out_ext = nc.dram_tensor("out", [ROWS, D], F32, kind="ExternalOutput").ap()

    with tile.TileContext(nc) as tc:
        _kernel_body(tc, nc, x_in, wt_in, wot_in, cos_in, sin_in, msk_in, out_ext)

    nc.compile()
    return nc


def _kernel_body(tc, nc, x_in, wt_in, wot_in, cos_in, sin_in, msk_in, out_ext):
    K4 = S // 128  # 32 key chunks of 128
    ctx = ExitStack()

    consts = ctx.enter_context(tc.tile_pool(name="consts", bufs=1))
    sb = ctx.enter_context(tc.tile_pool(name="sb", bufs=2))
    ropep = ctx.enter_context(tc.tile_pool(name="ropep", bufs=3))
    ptp = ctx.enter_context(tc.tile_pool(name="ptp", bufs=4))
    epip = ctx.enter_context(tc.tile_pool(name="epip", bufs=2))
    dram = ctx.enter_context(tc.tile_pool(name="dram", bufs=1, space="DRAM"))
    # PSUM budget: pool_a 2 banks + pst 4 banks + pav 2 banks = 8 banks
    pool_a = ctx.enter_context(tc.tile_pool(name="pool_a", bufs=2, space="PSUM"))
    pst = ctx.enter_context(tc.tile_pool(name="pst", bufs=2, space="PSUM"))
    pav = ctx.enter_context(tc.tile_pool(name="pav", bufs=1, space="PSUM"))

    # ---- constants / weights resident in SBUF ----
    ident = consts.tile([128, 128], BF16)
    make_identity(nc, ident)
    ones_row = consts.tile([1, 64], F32)
    nc.vector.memset(ones_row[:], 1.0)

    wt_sb = consts.tile([128, 8, 3 * HPC * DH], BF16)
    nc.sync.dma_start(wt_sb[:], wt_in.rearrange("(o p) m -> p o m", p=128))
    wot_sb = consts.tile([128, 8, D], BF16)
    nc.sync.dma_start(wot_sb[:], wot_in.rearrange("(o p) m -> p o m", p=128))
    cos_sb = consts.tile([128, S], BF16)
    nc.sync.dma_start(cos_sb[:], cos_in[:])
    sin_sb = consts.tile([128, S], BF16)
    nc.sync.dma_start(sin_sb[:], sin_in[:])
    msk_sb = consts.tile([128, 4, 2, W], BF16)
    nc.sync.dma_start(msk_sb[:], msk_in.rearrange("p (a b f) -> p a b f", a=4, b=2))

    # persistent per-core activations
    qT = consts.tile([128, S], BF16)       # [2 heads x 64dh (e|o perm), S]
    kT = consts.tile([128, S], BF16)
    vex = consts.tile([128, K4, 130], BF16)  # [s-in, s-out, v_h0 |1| v_h1 |1]
    nc.vector.memset(vex[:, :, 64:65], 1.0)
    nc.vector.memset(vex[:, :, 129:130], 1.0)
    yT = consts.tile([128, S], BF16)       # normalized attention out^T

    for w in range(NW):
        ws = slice(w * W, (w + 1) * W)
        # ---- load x window, transpose to xT via PE (bf16) ----
        xin = sb.tile([128, 4, D], BF16, name="xin")
        nc.sync.dma_start(
            xin[:], x_in[w * W:(w + 1) * W, :].rearrange("(a p) d -> p a d", p=128))
        xT = sb.tile([128, 8, W], BF16, name="xT")
        for d in range(8):
            tp = pool_a.tile([128, W], BF16, name="tp", tag="a")
            for s4 in range(4):
                nc.tensor.transpose(tp[:, s4 * 128:(s4 + 1) * 128],
                                    xin[:, s4, d * 128:(d + 1) * 128], ident)
            nc.vector.tensor_copy(xT[:, d, :], tp[:])

        # ---- QKV projection ----
        ev_s = ropep.tile([128, W], BF16, name="ev_s")
        od_s = ropep.tile([128, W], BF16, name="od_s")
        vT = ropep.tile([128, W], BF16, name="vT")
        for t, dst in ((0, ev_s), (1, od_s), (2, vT)):
            pq = pool_a.tile([128, W], F32, name="pq", tag="a")
            for d in range(8):
                nc.tensor.matmul(pq[:], wt_sb[:, d, t * 128:(t + 1) * 128],
                                 xT[:, d, :], start=(d == 0), stop=(d == 7))
            nc.vector.tensor_copy(dst[:], pq[:])
        # v: transpose to seq-major, pack into vex (ones cols preset)
        ptv = pool_a.tile([128, W], BF16, name="ptv", tag="a")
        for b in range(4):
            nc.tensor.transpose(ptv[:, b * 128:(b + 1) * 128],
                                vT[:, b * 128:(b + 1) * 128], ident)
        ptv4 = ptv.rearrange("p (b d) -> p b d", b=4)
        nc.vector.tensor_copy(vex[:, w * 4:(w + 1) * 4, 0:64], ptv4[:, :, 0:64])
        nc.vector.tensor_copy(vex[:, w * 4:(w + 1) * 4, 65:129], ptv4[:, :, 64:128])

        # ---- RoPE ----
        cw = cos_sb[:, ws]
        sw = sin_sb[:, ws]
        t1 = ropep.tile([128, W], BF16, name="t1")
        t2 = ropep.tile([128, W], BF16, name="t2")
        re = ropep.tile([128, W], BF16, name="re")
        ro = ropep.tile([128, W], BF16, name="ro")
        nc.vector.tensor_tensor(t1[:], ev_s[:], cw, mybir.AluOpType.mult)
        nc.vector.tensor_tensor(t2[:], od_s[:], sw, mybir.AluOpType.mult)
        nc.vector.tensor_tensor(re[:], t1[:], t2[:], mybir.AluOpType.subtract)
        nc.vector.tensor_tensor(t1[:], ev_s[:], sw, mybir.AluOpType.mult)
        nc.vector.tensor_tensor(t2[:], od_s[:], cw, mybir.AluOpType.mult)
        nc.vector.tensor_tensor(ro[:], t1[:], t2[:], mybir.AluOpType.add)
        for h in range(2):
            nc.vector.tensor_copy(qT[h * 64:h * 64 + 32, ws],
                                  re[h * 32:(h + 1) * 32, :])
            nc.vector.tensor_copy(qT[h * 64 + 32:h * 64 + 64, ws],
                                  ro[h * 32:(h + 1) * 32, :])
            nc.vector.tensor_copy(kT[h * 64:h * 64 + 32, ws],
                                  re[64 + h * 32:64 + (h + 1) * 32, :])
            nc.vector.tensor_copy(kT[h * 64 + 32:h * 64 + 64, ws],
                                  ro[64 + h * 32:64 + (h + 1) * 32, :])

        # ---- attention for window w (keys 0 .. (w+1)*512), both heads ----
        nk = 4 * (w + 1)
        av0 = pav.tile([65, W], F32, name="av0")
        av1 = pav.tile([65, W], F32, name="av1")
        avs = (av0, av1)
        for j in range(nk):
            stp = pst.tile([128, 2, W], F32, name="stp")
            for h in range(2):
                hs = slice(h * 64, (h + 1) * 64)
                nc.tensor.matmul(stp[:, h, :], kT[hs, j * 128:(j + 1) * 128],
                                 qT[hs, ws], start=True, stop=True)
            pt = ptp.tile([128, 2, W], BF16, name="pt")
            nc.scalar.activation(pt[:], stp[:],
                                 mybir.ActivationFunctionType.Exp,
                                 scale=1.0 / math.sqrt(DH))
            jl = j - 4 * w
            if jl >= 0:  # diagonal chunk: causal mask (both heads at once)
                nc.vector.tensor_tensor(pt[:], pt[:], msk_sb[:, jl],
                                        mybir.AluOpType.mult)
            for h in range(2):
                nc.tensor.matmul(avs[h][:], vex[:, j, h * 65:(h + 1) * 65],
                                 pt[:, h, :],
                                 start=(j == 0), stop=(j == nk - 1))
        # epilogue: normalize by rowsum (row 64 of av)
        for h in range(2):
            hs = slice(h * 64, (h + 1) * 64)
            rec = epip.tile([1, W], F32, name="rec")
            nc.vector.reciprocal(rec[:], avs[h][64:65, :])
            rb = pool_a.tile([64, W], F32, name="rb", tag="a")
            nc.tensor.matmul(rb[:], ones_row[:], rec[:], start=True, stop=True)
            rbs = epip.tile([64, W], F32, name="rbs")
            nc.vector.tensor_copy(rbs[:], rb[:])
            nc.vector.tensor_tensor(yT[hs, ws], avs[h][0:64, :], rbs[:],
                                    mybir.AluOpType.mult)

    # ---- AllToAll: head-sharded Y^T -> sequence-sharded ----
    # send block c' = yT[:, c'*512:(c'+1)*512]; receive block c' = core c''s
    # yT for MY row range -> ya2a rows c'*128+p = global Y column index.
    yt_dram = dram.tile([128 * N_CORES, ROWS], BF16)
    ya2a = dram.tile([128 * N_CORES, ROWS], BF16)
    for c in range(N_CORES):
        nc.sync.dma_start(yt_dram[c * 128:(c + 1) * 128, :],
                          yT[:, c * ROWS:(c + 1) * ROWS])
    nc.gpsimd.collective_compute(
        "AllToAll", mybir.AluOpType.bypass,
        replica_groups=[list(range(N_CORES))],
        ins=[yt_dram.opt()], outs=[ya2a.opt()])

    # ---- output projection for this core's 512-row block ----
    yg_sb = sb.tile([128, 8, ROWS], BF16, name="yg_sb")
    for yc in range(8):
        nc.sync.dma_start(yg_sb[:, yc, :], ya2a[yc * 128:(yc + 1) * 128, :])
    out_sb = sb.tile([128, 4, D], F32, name="out_sb")
    for r in range(4):
        for nh in range(2):
            po = pool_a.tile([128, W], F32, name="po", tag="a")
            for yc in range(8):
                nc.tensor.matmul(po[:], yg_sb[:, yc, r * 128:(r + 1) * 128],
                                 wot_sb[:, yc, nh * W:(nh + 1) * W],
                                 start=(yc == 0), stop=(yc == 7))
            nc.vector.tensor_copy(out_sb[:, r, nh * W:(nh + 1) * W], po[:])
    nc.sync.dma_start(out_ext.rearrange("(a p) m -> p a m", p=128), out_sb[:])

    ctx.close()


_NC_CACHE = None


def _host_inputs(x, Wproj, Wo):
    """Host-side per-core input arrays (weight layout prep + tables)."""
    bf = ml_dtypes.bfloat16
    x_bf = np.ascontiguousarray(x).astype(bf)

    invf = 1.0 / 10000.0 ** (np.arange(0, DH, 2) / DH)
    ang = np.outer(invf, np.arange(S))  # [32, S]
    cos_t = np.ascontiguousarray(np.tile(np.cos(ang), (4, 1))).astype(bf)
    sin_t = np.ascontiguousarray(np.tile(np.sin(ang), (4, 1))).astype(bf)

    p = np.arange(128)[:, None]
    f = np.arange(W)[None, :]
    msk = np.concatenate(
        [np.tile((128 * jl + p <= f).astype(np.float32), (1, 2))
         for jl in range(4)], axis=1).astype(bf)  # [128, 4*2*512]

    wot = np.ascontiguousarray(Wo.T).astype(bf)  # [D, D]

    in_maps = []
    for c in range(N_CORES):
        h0, h1 = 2 * c, 2 * c + 1
        Wq = [Wproj[64 * h:64 * h + 64, :] for h in (h0, h1)]
        Wk = [Wproj[1024 + 64 * h:1024 + 64 * h + 64, :] for h in (h0, h1)]
        Wv = [Wproj[2048 + 64 * h:2048 + 64 * h + 64, :] for h in (h0, h1)]
        evens = np.concatenate([Wq[0][::2], Wq[1][::2], Wk[0][::2], Wk[1][::2]], 0)
        odds = np.concatenate([Wq[0][1::2], Wq[1][1::2], Wk[0][1::2], Wk[1][1::2]], 0)
        vs = np.concatenate([Wv[0], Wv[1]], 0)
        wt = np.ascontiguousarray(
            np.concatenate([evens, odds, vs], 0).T).astype(bf)  # [1024, 384]
        in_maps.append({
            "x": x_bf, "wt": wt, "wot": wot,
            "cos": cos_t, "sin": sin_t, "msk": msk,
        })
    return in_maps


def kernel(x, mask, Wproj, Wo):
    global _NC_CACHE
    if _NC_CACHE is None:
        _NC_CACHE = build_graph()
    nc = _NC_CACHE
    in_maps = _host_inputs(np.asarray(x), np.asarray(Wproj), np.asarray(Wo))
    res = run_bass_kernel_spmd(nc, in_maps, core_ids=list(range(N_CORES)))
    out = np.concatenate([res.results[c]["out"] for c in range(N_CORES)], axis=0)
    return np.ascontiguousarray(out.astype(np.float32))



# revision 8
# speedup vs baseline: 1.2876x; 1.2876x over previous
"""Distributed causal attention kernel for Trainium2 (8 NeuronCores), v4.

Tensor-parallel over heads: core c owns heads {2c, 2c+1}. Per core:
QKV projection from host-transposed x (bf16), RoPE fused into the
PSUM->SBUF drain, bf16 causal attention in the keys-on-partitions
layout. Softmax exp is split: 3/4 of key-chunk pairs on the Scalar
engine (native Exp), 1/4 on the Vector engine via an int16 Schraudolph
bit-trick writing bf16 weight bit patterns directly. Window w+1's QKV
is issued before window w's epilogue so the Tensor engine never drains
(p-state stays high). Two batched AllToAlls re-shard Y^T (64 output
rows per core per window); output projection runs per window-pair,
overlapped, with only the final exchange + one projection as tail.
"""
import sys

if "/opt/trn_rl_repo" not in sys.path:
    sys.path.insert(0, "/opt/trn_rl_repo")

import math
from contextlib import ExitStack

import numpy as np
import ml_dtypes

import concourse.bass as bass
import concourse.bacc as bacc
import concourse.tile as tile
import concourse.mybir as mybir
from concourse.bass_utils import run_bass_kernel_spmd
from concourse.masks import make_identity

BF16 = mybir.dt.bfloat16
F32 = mybir.dt.float32
FP8 = mybir.dt.float8e4
I16 = mybir.dt.int16
DR = mybir.MatmulPerfMode.DoubleRow
MUL = mybir.AluOpType.mult
ADD = mybir.AluOpType.add
SUB = mybir.AluOpType.subtract

N_CORES = 8
S = 4096
D = 1024
H = 16
DH = 64
W = 512                     # seq window
NW = S // W                 # windows = 8
ROWS = S // N_CORES         # output rows per core (64 per window)

# Schraudolph exp -> bf16 bit pattern: I16 = A*s + B, truncated.
SCH_A = 128 * 1.44269504 * 0.125
SCH_B = 16249.0
DVE_PAIR_MOD = 4            # every 4th chunk-pair's exp on DVE (bf16 path)


def build_graph():
    nc = bacc.Bacc("TRN2", target_bir_lowering=False, debug=False,
                   num_devices=N_CORES)

    xt_in = nc.dram_tensor("xt", [D, S], BF16, kind="ExternalInput").ap()
    wt_in = nc.dram_tensor("wt", [D, 3 * 128], BF16, kind="ExternalInput").ap()
    wot_in = nc.dram_tensor("wot", [D, D], BF16, kind="ExternalInput").ap()
    cos_in = nc.dram_tensor("cos", [128, S], BF16, kind="ExternalInput").ap()
    sin_in = nc.dram_tensor("sin", [128, S], BF16, kind="ExternalInput").ap()
    mskb_in = nc.dram_tensor("mskb", [128, 4 * 2 * W], BF16, kind="ExternalInput").ap()
    out_ext = nc.dram_tensor("out", [ROWS, D], F32, kind="ExternalOutput").ap()

    with tile.TileContext(nc) as tc:
        _kernel_body(tc, nc, xt_in, wt_in, wot_in, cos_in, sin_in,
                     mskb_in, out_ext)

    nc.compile()
    return nc


def _kernel_body(tc, nc, xt_in, wt_in, wot_in, cos_in, sin_in,
                 mskb_in, out_ext):
    ctx = ExitStack()

    consts = ctx.enter_context(tc.tile_pool(name="consts", bufs=1))
    sbx = ctx.enter_context(tc.tile_pool(name="sbx", bufs=3))
    ropep = ctx.enter_context(tc.tile_pool(name="ropep", bufs=2))
    ptp = ctx.enter_context(tc.tile_pool(name="ptp", bufs=3))
    ptbp = ctx.enter_context(tc.tile_pool(name="ptbp", bufs=2))
    epip = ctx.enter_context(tc.tile_pool(name="epip", bufs=2))
    projp = ctx.enter_context(tc.tile_pool(name="projp", bufs=2))
    dram = ctx.enter_context(tc.tile_pool(name="dram", bufs=1, space="DRAM"))
    # PSUM: pool_a 2x1 bank + pst 2x2 banks + pav 1x2 banks = 8 banks
    pool_a = ctx.enter_context(tc.tile_pool(name="pool_a", bufs=2, space="PSUM"))
    pst = ctx.enter_context(tc.tile_pool(name="pst", bufs=2, space="PSUM"))
    pav = ctx.enter_context(tc.tile_pool(name="pav", bufs=1, space="PSUM"))

    # ---- constants / weights resident in SBUF ----
    ident = consts.tile([128, 128], BF16)
    make_identity(nc, ident)
    ones_row = consts.tile([1, 64], BF16)
    nc.vector.memset(ones_row[:], 1.0)

    wt_sb = consts.tile([128, 8, 3 * 128], BF16)
    nc.sync.dma_start(wt_sb[:], wt_in.rearrange("(o p) m -> p o m", p=128))
    cos_sb = consts.tile([128, S], BF16)
    nc.sync.dma_start(cos_sb[:], cos_in[:])
    sin_sb = consts.tile([128, S], BF16)
    nc.sync.dma_start(sin_sb[:], sin_in[:])
    mskb_sb = consts.tile([128, 4, 2, W], BF16)
    nc.sync.dma_start(mskb_sb[:], mskb_in.rearrange("p (a b f) -> p a b f", a=4, b=2))
    wot_sb = consts.tile([128, 8, D], BF16)

    # persistent per-core activations: [h0 even(32)|odd(32) ; h1 ...], bf16
    qT = consts.tile([128, S], BF16)
    kT = consts.tile([128, S], BF16)
    # v seq-major: [key, chunk, 160]: head h's v at cols 80h..80h+64,
    # rowsum-ones column at col 80h+64
    vexb = consts.tile([128, S // 128, 160], BF16)
    nc.vector.memset(vexb[:, :, 64:65], 1.0)
    nc.vector.memset(vexb[:, :, 144:145], 1.0)
    yT = consts.tile([128, S], BF16)       # normalized attention out^T

    ytwA = dram.tile([128 * N_CORES, 6 * 64], BF16, name="ytwA")
    ytwB = dram.tile([128 * N_CORES, 2 * 64], BF16, name="ytwB")
    ygaA = dram.tile([128 * N_CORES, 6 * 64], BF16, name="ygaA")
    ygaB = dram.tile([128 * N_CORES, 2 * 64], BF16, name="ygaB")

    xw = [None] * NW

    def dma_xw(w):
        xw[w] = sbx.tile([128, 8, W], BF16, name="xw", tag="xw")
        nc.sync.dma_start(
            xw[w][:],
            xt_in[:, w * W:(w + 1) * W].rearrange("(o p) m -> p o m", p=128))

    dma_xw(0)
    dma_xw(1)
    nc.sync.dma_start(wot_sb[:], wot_in.rearrange("(o p) m -> p o m", p=128))

    pair_cnt = [0]

    def qkv_rope(w):
        """QKV projection + RoPE for window w -> q8/kk, vex/vexb."""
        ws = slice(w * W, (w + 1) * W)
        pev = pool_a.tile([128, W], F32, name="pev", tag="a")
        pod = pool_a.tile([128, W], F32, name="pod", tag="a")
        for t, dst in ((0, pev), (1, pod)):
            for d in range(8):
                nc.tensor.matmul(dst[:], wt_sb[:, d, t * 128:(t + 1) * 128],
                                 xw[w][:, d, :], start=(d == 0), stop=(d == 7))
        cw = cos_sb[:, ws]
        sw = sin_sb[:, ws]
        t1 = ropep.tile([128, W], BF16, name="t1")
        t2 = ropep.tile([128, W], BF16, name="t2")
        t3 = ropep.tile([128, W], BF16, name="t3")
        t4 = ropep.tile([128, W], BF16, name="t4")
        nc.vector.tensor_tensor(t1[:], pev[:], cw, MUL)
        nc.vector.tensor_tensor(t2[:], pod[:], sw, MUL)
        nc.vector.tensor_tensor(t3[:], pev[:], sw, MUL)
        nc.vector.tensor_tensor(t4[:], pod[:], cw, MUL)
        # e' = e*cos - o*sin ; o' = e*sin + o*cos into qT/kT
        # layout: head h rows 64h..64h+64 = [even'(32); odd'(32)]
        for h in range(2):
            hr = slice(32 * h, 32 * h + 32)
            nc.gpsimd.tensor_tensor(qT[64 * h:64 * h + 32, ws], t1[hr, :], t2[hr, :], SUB)
            nc.gpsimd.tensor_tensor(qT[64 * h + 32:64 * h + 64, ws], t3[hr, :], t4[hr, :], ADD)
            kr = slice(64 + 32 * h, 64 + 32 * h + 32)
            nc.gpsimd.tensor_tensor(kT[64 * h:64 * h + 32, ws], t1[kr, :], t2[kr, :], SUB)
            nc.gpsimd.tensor_tensor(kT[64 * h + 32:64 * h + 64, ws], t3[kr, :], t4[kr, :], ADD)

        # v: project, transpose to seq-major, pack fp8 pairs + bf16 copy
        pv = pool_a.tile([128, W], F32, name="pv", tag="a")
        for d in range(8):
            nc.tensor.matmul(pv[:], wt_sb[:, d, 256:384], xw[w][:, d, :],
                             start=(d == 0), stop=(d == 7))
        vTsb = ropep.tile([128, W], BF16, name="vTsb")
        nc.vector.tensor_copy(vTsb[:], pv[:])
        ptv = pool_a.tile([128, W], BF16, name="ptv", tag="a")
        for b in range(4):
            nc.tensor.transpose(ptv[:, b * 128:(b + 1) * 128],
                                vTsb[:, b * 128:(b + 1) * 128], ident)
        ptv2 = ptv.rearrange("p (j d) -> p j d", j=4)
        nc.vector.tensor_copy(vexb[:, 4 * w:4 * w + 4, 0:64], ptv2[:, :, 0:64])
        nc.vector.tensor_copy(vexb[:, 4 * w:4 * w + 4, 80:144], ptv2[:, :, 64:128])

    def jloop(w, av):
        """Scores/exp/AV for window w's queries, both heads -> av."""
        ws = slice(w * W, (w + 1) * W)
        npair = 2 * (w + 1)
        for jp in range(npair):
            on_dve = pair_cnt[0] % DVE_PAIR_MOD == DVE_PAIR_MOD - 1
            pair_cnt[0] += 1
            pt = ptp.tile([128, 2, 2, W], BF16, name="pt")
            for i in range(2):
                j = 2 * jp + i
                stp = pst.tile([128, 2, W], F32, name="stp", tag="s")
                for h in range(2):
                    hs = slice(64 * h, 64 * h + 64)
                    nc.tensor.matmul(stp[:, h, :],
                                     kT[hs, j * 128:(j + 1) * 128],
                                     qT[hs, ws], start=True, stop=True)
                if on_dve:
                    nc.vector.tensor_scalar(pt[:, i, :, :].bitcast(I16), stp[:],
                                            SCH_A, SCH_B, MUL, ADD)
                else:
                    nc.scalar.activation(pt[:, i, :, :], stp[:],
                                         mybir.ActivationFunctionType.Exp,
                                         scale=0.125)
                jl = j - 4 * w
                if jl >= 0:  # diagonal chunk: causal mask (both heads)
                    if on_dve:
                        nc.vector.tensor_tensor(pt[:, i, :, :], pt[:, i, :, :],
                                                mskb_sb[:, jl], MUL)
                    else:
                        nc.gpsimd.tensor_tensor(pt[:, i, :, :], pt[:, i, :, :],
                                                mskb_sb[:, jl], MUL)
            for h in range(2):
                for i in range(2):
                    j = 2 * jp + i
                    nc.tensor.matmul(av[:, h, :],
                                     vexb[:, j, 80 * h:80 * h + 65],
                                     pt[:, i, h, :],
                                     start=(jp == 0 and i == 0),
                                     stop=(jp == npair - 1 and i == 1))

    def epilogue(w, av):
        """Normalize by rowsum (row 64 of av) -> yT."""
        ws = slice(w * W, (w + 1) * W)
        s1 = epip.tile([1, 2, W], BF16, name="s1")
        nc.vector.tensor_copy(s1[:], av[64:65, :, :])
        brec = pst.tile([64, 2, W], F32, name="brec", tag="s")
        for h in range(2):
            nc.tensor.matmul(brec[:, h, :], ones_row[:], s1[:, h, :],
                             start=True, stop=True)
        rbs = epip.tile([64, 2, W], F32, name="rbs")
        nc.vector.reciprocal_approx_fast(rbs[:], brec[:])
        nc.vector.tensor_tensor(yT[0:64, ws], av[0:64, 0, :], rbs[:, 0, :], MUL)
        nc.vector.tensor_tensor(yT[64:128, ws], av[0:64, 1, :], rbs[:, 1, :], MUL)

    def stage(w):
        """Stage window w's yT block for the batched exchange."""
        ws = slice(w * W, (w + 1) * W)
        if w < 6:
            dst = ytwA[:, w * 64:(w + 1) * 64]
        else:
            dst = ytwB[:, (w - 6) * 64:(w - 5) * 64]
        nc.sync.dma_start(dst.rearrange("(c p) q -> p c q", p=128),
                          yT[:, ws].rearrange("p (c q) -> p c q", c=N_CORES))

    def proj(p):
        """Output projection for window pair (2p, 2p+1): my 128 rows."""
        yg2 = projp.tile([128, 8, 128], BF16, name="yg2")
        if p < 3:
            src = ygaA[:, 128 * p:128 * (p + 1)]
        else:
            src = ygaB[:, :]
        nc.sync.dma_start(yg2[:], src.rearrange("(o pp) q -> pp o q", pp=128))
        pout = pst.tile([128, 2, W], F32, name="pout", tag="s")
        for nh in range(2):
            for o in range(8):
                nc.tensor.matmul(pout[:, nh, :], yg2[:, o, :],
                                 wot_sb[:, o, nh * W:(nh + 1) * W],
                                 start=(o == 0), stop=(o == 7))
        osb = projp.tile([128, D], F32, name="osb")
        nc.vector.tensor_copy(osb[:], pout[:])
        nc.sync.dma_start(out_ext[128 * p:128 * (p + 1), :], osb[:])

    qkv_rope(0)
    for w in range(NW):
        av = pav.tile([65, 2, W], F32, name="av")
        jloop(w, av)
        if w + 2 < NW:
            dma_xw(w + 2)
        if w + 1 < NW:
            qkv_rope(w + 1)
        if w == 6:
            proj(0)
            proj(1)
        if w == 7:
            proj(2)
        epilogue(w, av)
        stage(w)
        if w == 5:
            nc.gpsimd.collective_compute(
                "AllToAll", mybir.AluOpType.bypass,
                replica_groups=[list(range(N_CORES))],
                ins=[ytwA.opt()], outs=[ygaA.opt()])
        if w == 7:
            nc.gpsimd.collective_compute(
                "AllToAll", mybir.AluOpType.bypass,
                replica_groups=[list(range(N_CORES))],
                ins=[ytwB.opt()], outs=[ygaB.opt()])
            proj(3)

    ctx.close()


_NC_CACHE = None


def _host_inputs(x, Wproj, Wo):
    """Host-side per-core input arrays (weight layout prep + tables)."""
    bf = ml_dtypes.bfloat16
    xt = np.ascontiguousarray(np.asarray(x).T).astype(bf)  # [D, S]

    invf = 1.0 / 10000.0 ** (np.arange(0, DH, 2) / DH)
    ang = np.outer(invf, np.arange(S))  # [32, S]
    cos_t = np.ascontiguousarray(np.tile(np.cos(ang), (4, 1))).astype(bf)
    sin_t = np.ascontiguousarray(np.tile(np.sin(ang), (4, 1))).astype(bf)

    p = np.arange(128)[:, None]
    f = np.arange(W)[None, :]
    msk_f = np.concatenate(
        [np.tile((128 * jl + p <= f).astype(np.float32), (1, 2))
         for jl in range(4)], axis=1)  # [128, 4*2*512]
    mskb = msk_f.astype(bf)

    wot = np.ascontiguousarray(Wo.T).astype(bf)  # [D, D]

    in_maps = []
    for c in range(N_CORES):
        h0, h1 = 2 * c, 2 * c + 1
        Wq = [Wproj[64 * h:64 * h + 64, :] for h in (h0, h1)]
        Wk = [Wproj[1024 + 64 * h:1024 + 64 * h + 64, :] for h in (h0, h1)]
        Wv = [Wproj[2048 + 64 * h:2048 + 64 * h + 64, :] for h in (h0, h1)]
        evens = np.concatenate([Wq[0][::2], Wq[1][::2], Wk[0][::2], Wk[1][::2]], 0)
        odds = np.concatenate([Wq[0][1::2], Wq[1][1::2], Wk[0][1::2], Wk[1][1::2]], 0)
        vs = np.concatenate([Wv[0], Wv[1]], 0)
        wt = np.ascontiguousarray(
            np.concatenate([evens, odds, vs], 0).T).astype(bf)  # [1024, 384]
        in_maps.append({
            "xt": xt, "wt": wt, "wot": wot,
            "cos": cos_t, "sin": sin_t, "mskb": mskb,
        })
    return in_maps


def kernel(x, mask, Wproj, Wo):
    global _NC_CACHE
    if _NC_CACHE is None:
        _NC_CACHE = build_graph()
    nc = _NC_CACHE
    in_maps = _host_inputs(np.asarray(x), np.asarray(Wproj), np.asarray(Wo))
    res = run_bass_kernel_spmd(nc, in_maps, core_ids=list(range(N_CORES)))
    out = np.empty((S, D), dtype=np.float32)
    for c in range(N_CORES):
        oc = res.results[c]["out"].astype(np.float32)  # [512, 1024]
        for w in range(NW):
            out[w * W + c * 64:w * W + (c + 1) * 64, :] = \
                oc[w * 64:(w + 1) * 64, :]
    return np.ascontiguousarray(out)


# revision 11
# speedup vs baseline: 1.4000x; 1.0873x over previous
"""Distributed causal attention kernel for Trainium2 (8 NeuronCores), v4.

Tensor-parallel over heads: core c owns heads {2c, 2c+1}. Per core:
QKV projection from host-transposed x (bf16), RoPE fused into the
PSUM->SBUF drain, bf16 causal attention in the keys-on-partitions
layout. Softmax exp is split: 3/4 of key-chunk pairs on the Scalar
engine (native Exp), 1/4 on the Vector engine via an int16 Schraudolph
bit-trick writing bf16 weight bit patterns directly. Window w+1's QKV
is issued before window w's epilogue so the Tensor engine never drains
(p-state stays high). Two batched AllToAlls re-shard Y^T (64 output
rows per core per window); output projection runs per window-pair,
overlapped, with only the final exchange + one projection as tail.
"""
import sys

if "/opt/trn_rl_repo" not in sys.path:
    sys.path.insert(0, "/opt/trn_rl_repo")

import math
from contextlib import ExitStack

import numpy as np
import ml_dtypes

import concourse.bass as bass
import concourse.bacc as bacc
import concourse.tile as tile
import concourse.mybir as mybir
from concourse.bass_utils import run_bass_kernel_spmd
from concourse.masks import make_identity

BF16 = mybir.dt.bfloat16
F32 = mybir.dt.float32
FP8 = mybir.dt.float8e4
I16 = mybir.dt.int16
DR = mybir.MatmulPerfMode.DoubleRow
MUL = mybir.AluOpType.mult
ADD = mybir.AluOpType.add
SUB = mybir.AluOpType.subtract

N_CORES = 8
S = 4096
D = 1024
H = 16
DH = 64
W = 512                     # seq window
NW = S // W                 # windows = 8
ROWS = S // N_CORES         # output rows per core (64 per window)

# Schraudolph exp -> bf16 bit pattern: I16 = A*s + B, truncated.
SCH_A = 128 * 1.44269504 * 0.125
SCH_B = 16249.0
DVE_PAIR_MOD = 4            # every 4th chunk-pair's exp on DVE (bf16 path)


def build_graph():
    nc = bacc.Bacc("TRN2", target_bir_lowering=False, debug=False,
                   num_devices=N_CORES)

    xt_in = nc.dram_tensor("xt", [D, S], BF16, kind="ExternalInput").ap()
    wt_in = nc.dram_tensor("wt", [D, 3 * 128], BF16, kind="ExternalInput").ap()
    wot_in = nc.dram_tensor("wot", [D, D], BF16, kind="ExternalInput").ap()
    cos_in = nc.dram_tensor("cos", [128, S], BF16, kind="ExternalInput").ap()
    sin_in = nc.dram_tensor("sin", [128, S], BF16, kind="ExternalInput").ap()
    mskb_in = nc.dram_tensor("mskb", [128, 4 * 2 * W], BF16, kind="ExternalInput").ap()
    out_ext = nc.dram_tensor("out", [ROWS, D], F32, kind="ExternalOutput").ap()

    with tile.TileContext(nc) as tc:
        _kernel_body(tc, nc, xt_in, wt_in, wot_in, cos_in, sin_in,
                     mskb_in, out_ext)

    nc.compile()
    return nc


def _kernel_body(tc, nc, xt_in, wt_in, wot_in, cos_in, sin_in,
                 mskb_in, out_ext):
    ctx = ExitStack()

    consts = ctx.enter_context(tc.tile_pool(name="consts", bufs=1))
    sbx = ctx.enter_context(tc.tile_pool(name="sbx", bufs=3))
    ropep = ctx.enter_context(tc.tile_pool(name="ropep", bufs=2))
    ptp = ctx.enter_context(tc.tile_pool(name="ptp", bufs=3))
    ptbp = ctx.enter_context(tc.tile_pool(name="ptbp", bufs=2))
    epip = ctx.enter_context(tc.tile_pool(name="epip", bufs=2))
    projp = ctx.enter_context(tc.tile_pool(name="projp", bufs=2))
    dram = ctx.enter_context(tc.tile_pool(name="dram", bufs=1, space="DRAM"))
    # PSUM: pool_a 2x1 bank + pst 2x2 banks + pav 1x2 banks = 8 banks
    pool_a = ctx.enter_context(tc.tile_pool(name="pool_a", bufs=2, space="PSUM"))
    pst = ctx.enter_context(tc.tile_pool(name="pst", bufs=2, space="PSUM"))
    pav = ctx.enter_context(tc.tile_pool(name="pav", bufs=1, space="PSUM"))

    # ---- constants / weights resident in SBUF ----
    ident = consts.tile([128, 128], BF16)
    make_identity(nc, ident)
    ones_row = consts.tile([1, 64], BF16)
    nc.vector.memset(ones_row[:], 1.0)

    wt_sb = consts.tile([128, 8, 3 * 128], BF16)
    nc.sync.dma_start(wt_sb[:], wt_in.rearrange("(o p) m -> p o m", p=128))
    cos_sb = consts.tile([128, S], BF16)
    nc.sync.dma_start(cos_sb[:], cos_in[:])
    sin_sb = consts.tile([128, S], BF16)
    nc.sync.dma_start(sin_sb[:], sin_in[:])
    mskb_sb = consts.tile([128, 4, 2, W], BF16)
    nc.sync.dma_start(mskb_sb[:], mskb_in.rearrange("p (a b f) -> p a b f", a=4, b=2))
    wot_sb = consts.tile([128, 8, D], BF16)

    # persistent per-core activations: [h0 even(32)|odd(32) ; h1 ...], bf16
    qT = consts.tile([128, S], BF16)
    kT = consts.tile([128, S], BF16)
    # v seq-major: [key, chunk, 160]: head h's v at cols 80h..80h+64,
    # rowsum-ones column at col 80h+64
    vexb = consts.tile([128, S // 128, 160], BF16)
    nc.vector.memset(vexb[:, :, 64:65], 1.0)
    nc.vector.memset(vexb[:, :, 144:145], 1.0)
    yT = consts.tile([128, S], BF16)       # normalized attention out^T

    ytwA = dram.tile([128 * N_CORES, 6 * 64], BF16, name="ytwA")
    ytwB = dram.tile([128 * N_CORES, 2 * 64], BF16, name="ytwB")
    ygaA = dram.tile([128 * N_CORES, 6 * 64], BF16, name="ygaA")
    ygaB = dram.tile([128 * N_CORES, 2 * 64], BF16, name="ygaB")

    xw = [None] * NW

    def dma_xw(w):
        xw[w] = sbx.tile([128, 8, W], BF16, name="xw", tag="xw")
        nc.sync.dma_start(
            xw[w][:],
            xt_in[:, w * W:(w + 1) * W].rearrange("(o p) m -> p o m", p=128))

    dma_xw(0)
    dma_xw(1)
    nc.sync.dma_start(wot_sb[:], wot_in.rearrange("(o p) m -> p o m", p=128))

    pair_cnt = [0]

    def qkv_fillers(w):
        """QKV projection + RoPE for window w as a list of filler closures
        (interleaved between j-loop pairs to keep all engines fed)."""
        ws = slice(w * W, (w + 1) * W)
        st = {}

        def f_ev(lo, hi):
            if lo == 0:
                st["pev"] = pool_a.tile([128, W], F32, name="pev", tag="a")
            for d in range(lo, hi):
                nc.tensor.matmul(st["pev"][:], wt_sb[:, d, 0:128],
                                 xw[w][:, d, :], start=(d == 0), stop=(d == 7))

        def f_od(lo, hi):
            if lo == 0:
                st["pod"] = pool_a.tile([128, W], F32, name="pod", tag="a")
            for d in range(lo, hi):
                nc.tensor.matmul(st["pod"][:], wt_sb[:, d, 128:256],
                                 xw[w][:, d, :], start=(d == 0), stop=(d == 7))

        def f_t12():
            st["t1"] = ropep.tile([128, W], BF16, name="t1")
            st["t2"] = ropep.tile([128, W], BF16, name="t2")
            nc.vector.tensor_tensor(st["t1"][:], st["pev"][:], cos_sb[:, ws], MUL)
            nc.vector.tensor_tensor(st["t2"][:], st["pod"][:], sin_sb[:, ws], MUL)

        def f_t34():
            st["t3"] = ropep.tile([128, W], BF16, name="t3")
            st["t4"] = ropep.tile([128, W], BF16, name="t4")
            nc.vector.tensor_tensor(st["t3"][:], st["pev"][:], sin_sb[:, ws], MUL)
            nc.vector.tensor_tensor(st["t4"][:], st["pod"][:], cos_sb[:, ws], MUL)

        def f_qw():
            t1, t2, t3, t4 = st["t1"], st["t2"], st["t3"], st["t4"]
            for h in range(2):
                hr = slice(32 * h, 32 * h + 32)
                nc.gpsimd.tensor_tensor(qT[64 * h:64 * h + 32, ws], t1[hr, :], t2[hr, :], SUB)
                nc.gpsimd.tensor_tensor(qT[64 * h + 32:64 * h + 64, ws], t3[hr, :], t4[hr, :], ADD)

        def f_kw():
            t1, t2, t3, t4 = st["t1"], st["t2"], st["t3"], st["t4"]
            for h in range(2):
                kr = slice(64 + 32 * h, 64 + 32 * h + 32)
                nc.gpsimd.tensor_tensor(kT[64 * h:64 * h + 32, ws], t1[kr, :], t2[kr, :], SUB)
                nc.gpsimd.tensor_tensor(kT[64 * h + 32:64 * h + 64, ws], t3[kr, :], t4[kr, :], ADD)

        def f_pv(lo, hi):
            if lo == 0:
                st["pv"] = pool_a.tile([128, W], F32, name="pv", tag="a")
            for d in range(lo, hi):
                nc.tensor.matmul(st["pv"][:], wt_sb[:, d, 256:384],
                                 xw[w][:, d, :], start=(d == 0), stop=(d == 7))

        def f_vc():
            st["vTsb"] = ropep.tile([128, W], BF16, name="vTsb")
            nc.vector.tensor_copy(st["vTsb"][:], st["pv"][:])

        def f_tr():
            st["ptv"] = pool_a.tile([128, W], BF16, name="ptv", tag="a")
            for b in range(4):
                nc.tensor.transpose(st["ptv"][:, b * 128:(b + 1) * 128],
                                    st["vTsb"][:, b * 128:(b + 1) * 128], ident)

        def f_vx():
            ptv2 = st["ptv"].rearrange("p (j d) -> p j d", j=4)
            nc.vector.tensor_copy(vexb[:, 4 * w:4 * w + 4, 0:64], ptv2[:, :, 0:64])
            nc.vector.tensor_copy(vexb[:, 4 * w:4 * w + 4, 80:144], ptv2[:, :, 64:128])

        return [lambda: f_ev(0, 4), lambda: f_ev(4, 8),
                lambda: f_od(0, 4), lambda: f_od(4, 8),
                f_t12, f_t34, f_qw, f_kw,
                lambda: f_pv(0, 4), lambda: f_pv(4, 8),
                f_vc, f_tr, f_vx]

    def jloop(w, av, fillers):
        """Scores/exp/AV for window w's queries, both heads -> av."""
        npair = 2 * (w + 1)
        for jp in range(npair):
            on_dve = pair_cnt[0] % DVE_PAIR_MOD == DVE_PAIR_MOD - 1
            pair_cnt[0] += 1
            pt = ptp.tile([128, 2, 2, W], BF16, name="pt")
            for i in range(2):
                j = 2 * jp + i
                jl = j - 4 * w
                qo = max(jl, 0) * 128  # causal: queries < qo can't see chunk
                qs = slice(w * W + qo, (w + 1) * W)
                stp = pst.tile([128, 2, W], F32, name="stp", tag="s")
                for h in range(2):
                    hs = slice(64 * h, 64 * h + 64)
                    nc.tensor.matmul(stp[:, h, qo:W],
                                     kT[hs, j * 128:(j + 1) * 128],
                                     qT[hs, qs], start=True, stop=True)
                if on_dve:
                    nc.vector.tensor_scalar(pt[:, i, :, qo:W].bitcast(I16),
                                            stp[:, :, qo:W],
                                            SCH_A, SCH_B, MUL, ADD)
                else:
                    nc.scalar.activation(pt[:, i, :, qo:W], stp[:, :, qo:W],
                                         mybir.ActivationFunctionType.Exp,
                                         scale=0.125)
                if jl >= 0:  # diagonal chunk: causal mask (both heads)
                    if on_dve:
                        nc.vector.tensor_tensor(pt[:, i, :, qo:W],
                                                pt[:, i, :, qo:W],
                                                mskb_sb[:, jl, :, qo:W], MUL)
                    else:
                        nc.gpsimd.tensor_tensor(pt[:, i, :, qo:W],
                                                pt[:, i, :, qo:W],
                                                mskb_sb[:, jl, :, qo:W], MUL)
            for h in range(2):
                for i in range(2):
                    j = 2 * jp + i
                    qo = max(j - 4 * w, 0) * 128
                    nc.tensor.matmul(av[:, h, qo:W],
                                     vexb[:, j, 80 * h:80 * h + 65],
                                     pt[:, i, h, qo:W],
                                     start=(jp == 0 and i == 0),
                                     stop=(jp == npair - 1 and i == 1))
            if fillers:
                fillers.pop(0)()

    def epilogue(w, av):
        """Normalize by rowsum (row 64 of av) -> yT."""
        ws = slice(w * W, (w + 1) * W)
        s1 = epip.tile([1, 2, W], BF16, name="s1")
        nc.vector.tensor_copy(s1[:], av[64:65, :, :])
        brec = pst.tile([64, 2, W], F32, name="brec", tag="s")
        for h in range(2):
            nc.tensor.matmul(brec[:, h, :], ones_row[:], s1[:, h, :],
                             start=True, stop=True)
        rbs = epip.tile([64, 2, W], F32, name="rbs")
        nc.vector.reciprocal_approx_fast(rbs[:], brec[:])
        nc.vector.tensor_tensor(yT[0:64, ws], av[0:64, 0, :], rbs[:, 0, :], MUL)
        nc.vector.tensor_tensor(yT[64:128, ws], av[0:64, 1, :], rbs[:, 1, :], MUL)

    def stage(w):
        """Stage window w's yT block for the batched exchange."""
        ws = slice(w * W, (w + 1) * W)
        if w < 6:
            dst = ytwA[:, w * 64:(w + 1) * 64]
        else:
            dst = ytwB[:, (w - 6) * 64:(w - 5) * 64]
        nc.sync.dma_start(dst.rearrange("(c p) q -> p c q", p=128),
                          yT[:, ws].rearrange("p (c q) -> p c q", c=N_CORES))

    def proj(p):
        """Output projection for window pair (2p, 2p+1): my 128 rows."""
        yg2 = projp.tile([128, 8, 128], BF16, name="yg2")
        if p < 3:
            src = ygaA[:, 128 * p:128 * (p + 1)]
        else:
            src = ygaB[:, :]
        nc.sync.dma_start(yg2[:], src.rearrange("(o pp) q -> pp o q", pp=128))
        pout = pst.tile([128, 2, W], F32, name="pout", tag="s")
        for nh in range(2):
            for o in range(8):
                nc.tensor.matmul(pout[:, nh, :], yg2[:, o, :],
                                 wot_sb[:, o, nh * W:(nh + 1) * W],
                                 start=(o == 0), stop=(o == 7))
        osb = projp.tile([128, D], F32, name="osb")
        nc.vector.tensor_copy(osb[:], pout[:])
        nc.sync.dma_start(out_ext[128 * p:128 * (p + 1), :], osb[:])

    # window 0's QKV up front; later windows interleave into the j-loop
    for f in qkv_fillers(0):
        f()
    for w in range(NW):
        av = pav.tile([65, 2, W], F32, name="av")
        fillers = []
        if w + 1 < NW:
            fillers += qkv_fillers(w + 1)
        if w == 7:
            # delay projections until collectiveA has surely landed
            fillers += [(lambda: None)] * 13
            fillers += [lambda: proj(0), lambda: proj(1), lambda: proj(2)]
        jloop(w, av, fillers)
        for f in fillers:
            f()
        if w + 2 < NW:
            dma_xw(w + 2)
        if w == 6:
            nc.gpsimd.collective_compute(
                "AllToAll", mybir.AluOpType.bypass,
                replica_groups=[list(range(N_CORES))],
                ins=[ytwA.opt()], outs=[ygaA.opt()])
        epilogue(w, av)
        stage(w)
        if w == 7:
            nc.gpsimd.collective_compute(
                "AllToAll", mybir.AluOpType.bypass,
                replica_groups=[list(range(N_CORES))],
                ins=[ytwB.opt()], outs=[ygaB.opt()])
            proj(3)

    ctx.close()


_NC_CACHE = None


def _host_inputs(x, Wproj, Wo):
    """Host-side per-core input arrays (weight layout prep + tables)."""
    bf = ml_dtypes.bfloat16
    xt = np.ascontiguousarray(np.asarray(x).T).astype(bf)  # [D, S]

    invf = 1.0 / 10000.0 ** (np.arange(0, DH, 2) / DH)
    ang = np.outer(invf, np.arange(S))  # [32, S]
    cos_t = np.ascontiguousarray(np.tile(np.cos(ang), (4, 1))).astype(bf)
    sin_t = np.ascontiguousarray(np.tile(np.sin(ang), (4, 1))).astype(bf)

    p = np.arange(128)[:, None]
    f = np.arange(W)[None, :]
    msk_f = np.concatenate(
        [np.tile((128 * jl + p <= f).astype(np.float32), (1, 2))
         for jl in range(4)], axis=1)  # [128, 4*2*512]
    mskb = msk_f.astype(bf)

    wot = np.ascontiguousarray(Wo.T).astype(bf)  # [D, D]

    in_maps = []
    for c in range(N_CORES):
        h0, h1 = 2 * c, 2 * c + 1
        Wq = [Wproj[64 * h:64 * h + 64, :] for h in (h0, h1)]
        Wk = [Wproj[1024 + 64 * h:1024 + 64 * h + 64, :] for h in (h0, h1)]
        Wv = [Wproj[2048 + 64 * h:2048 + 64 * h + 64, :] for h in (h0, h1)]
        evens = np.concatenate([Wq[0][::2], Wq[1][::2], Wk[0][::2], Wk[1][::2]], 0)
        odds = np.concatenate([Wq[0][1::2], Wq[1][1::2], Wk[0][1::2], Wk[1][1::2]], 0)
        vs = np.concatenate([Wv[0], Wv[1]], 0)
        wt = np.ascontiguousarray(
            np.concatenate([evens, odds, vs], 0).T).astype(bf)  # [1024, 384]
        in_maps.append({
            "xt": xt, "wt": wt, "wot": wot,
            "cos": cos_t, "sin": sin_t, "mskb": mskb,
        })
    return in_maps


def kernel(x, mask, Wproj, Wo):
    global _NC_CACHE
    if _NC_CACHE is None:
        _NC_CACHE = build_graph()
    nc = _NC_CACHE
    in_maps = _host_inputs(np.asarray(x), np.asarray(Wproj), np.asarray(Wo))
    res = run_bass_kernel_spmd(nc, in_maps, core_ids=list(range(N_CORES)))
    out = np.empty((S, D), dtype=np.float32)
    for c in range(N_CORES):
        oc = res.results[c]["out"].astype(np.float32)  # [512, 1024]
        for w in range(NW):
            out[w * W + c * 64:w * W + (c + 1) * 64, :] = \
                oc[w * 64:(w + 1) * 64, :]
    return np.ascontiguousarray(out)
